# revision 1
# baseline (speedup 1.0000x reference)
"""CLUBMean loss kernel for Trainium2, 8-core data-parallel.

Math: with x_vec = mean_s(x), y_vec = mean_s(y), mu = MLP(x_vec):
  positive_i = -||mu_i - y_i||^2 / 2
  negative_i = -mean_j ||y_j - mu_i||^2 / 2
             = -(S2/N - 2 mu_i . Ey + ||mu_i||^2) / 2      (exact expansion)
  loss = mean_i(positive_i - negative_i)

Each core handles 128 of the 1024 samples and emits partial sums:
  out_vec (128,4): columns [Ey lo, Ey hi, Mu lo, Mu hi] summed over its samples
  out_row (1,3,128): per-sample ||mu-y||^2, ||mu||^2, ||y||^2
The host all-reduces the partials in float64 and applies the closed form.
"""

import sys

sys.path.insert(0, "/opt/trn_rl_repo")

from contextlib import ExitStack

import numpy as np

import concourse.bass as bass
import concourse.mybir as mybir
from concourse.bass_utils import run_bass_kernel_spmd
from concourse.masks import make_identity

N = 1024
P = 128            # samples per core
XC, YC, HID, S = 512, 256, 512, 64
CH = 64            # channel chunk per streamed DMA
NX = XC // CH      # 8 x chunks
NY = YC // CH      # 4 y chunks
NCHUNK = NX + NY   # 12
NBUF = 6           # stream buffer ring
NXV = 4            # pooled-vector ring
F32 = mybir.dt.float32
AX = mybir.AxisListType
ALU = mybir.AluOpType
ACTF = mybir.ActivationFunctionType

_CACHE = {}


def build_nc():
    nc = bass.Bass()
    x = nc.dram_tensor("x", [P, XC, S], F32, kind="ExternalInput")
    y = nc.dram_tensor("y", [P, YC, S], F32, kind="ExternalInput")
    w1 = nc.dram_tensor("w1", [XC, HID], F32, kind="ExternalInput")
    b1 = nc.dram_tensor("b1", [P, 4], F32, kind="ExternalInput")
    w2 = nc.dram_tensor("w2", [HID, YC], F32, kind="ExternalInput")
    b2 = nc.dram_tensor("b2", [P, 2], F32, kind="ExternalInput")
    out_vec = nc.dram_tensor("out_vec", [P, 4], F32, kind="ExternalOutput")
    out_row = nc.dram_tensor("out_row", [1, 3, P], F32, kind="ExternalOutput")

    ctx = ExitStack()
    with ctx:
        sb = lambda name, shape: ctx.enter_context(nc.sbuf_tensor(name, shape, F32))
        ps = lambda name, shape: ctx.enter_context(nc.psum_tensor(name, shape, F32))
        sem = lambda name: ctx.enter_context(nc.semaphore(name))

        xbuf = [sb(f"xbuf{i}", [P, CH, S]) for i in range(NBUF)]
        xv = [sb(f"xv{i}", [P, CH]) for i in range(NXV)]
        xvT = sb("xvT", [P, 4, P])
        yvT = sb("yvT", [P, 2, P])
        hT = sb("hT", [P, 4, P])
        muT = sb("muT", [P, 2, P])
        dtmp = sb("dtmp", [P, P])
        sqd = sb("sqd", [P, 2, P])
        sqmu = sb("sqmu", [P, 2, P])
        sqy = sb("sqy", [P, 2, P])
        w1sb = [sb(f"w1sb{k}", [P, HID]) for k in range(4)]
        w2sb = [sb(f"w2sb{k}", [P, YC]) for k in range(4)]
        b1sb = sb("b1sb", [P, 4])
        b2sb = sb("b2sb", [P, 2])
        ident = sb("ident", [P, P])
        ones = sb("ones", [P, 1])
        stat = sb("stat", [P, 4])
        rows = sb("rows", [1, 3, P])

        pt = [ps(f"pt{i}", [CH, P]) for i in range(2)]
        ph = ps("ph", [P, 4, P])
        pmu = ps("pmu", [P, 2, P])
        prow = ps("prow", [1, 3, P])

        dch = [sem(f"dch{i}") for i in range(NCHUNK)]
        dw = sem("dw")
        dout = sem("dout")
        s_const = sem("s_const")
        s_pool = sem("s_pool")
        s_tp = sem("s_tp")
        s_cp = sem("s_cp")
        s_hmm = sem("s_hmm")
        s_relu = sem("s_relu")
        s_mumm = sem("s_mumm")
        s_mubias = sem("s_mubias")
        s_sq = sem("s_sq")
        s_stat = sem("s_stat")
        s_rows = sem("s_rows")
        s_rowscp = sem("s_rowscp")

        with nc.Block() as block:

            @block.sync
            def _(e):
                # main input stream: x chunks 0..7 then y chunks 8..11
                for i in range(NCHUNK):
                    if i >= NBUF:
                        e.wait_ge(s_pool, i - NBUF + 1)
                    if i < NX:
                        src = x[:, i * CH:(i + 1) * CH, :]
                    else:
                        j = i - NX
                        src = y[:, j * CH:(j + 1) * CH, :]
                    e.dma_start(out=xbuf[i % NBUF][:, :, :], in_=src).then_inc(
                        dch[i], 16
                    )
                e.wait_ge(s_stat, 1)
                e.dma_start(out=out_vec[:, :], in_=stat[:, :]).then_inc(dout, 16)
                e.wait_ge(s_rowscp, 1)
                e.dma_start(out=out_row[:, :, :], in_=rows[:, :, :]).then_inc(
                    dout, 16
                )
                e.wait_ge(dout, 32)

            @block.gpsimd
            def _(e):
                make_identity(nc, ident[:, :])
                e.memset(ones[:, :], 1.0).then_inc(s_const, 1)
                for k in range(4):
                    e.dma_start(
                        out=w1sb[k][:, :], in_=w1[k * P:(k + 1) * P, :]
                    ).then_inc(dw, 16)
                for k in range(4):
                    e.dma_start(
                        out=w2sb[k][:, :], in_=w2[k * P:(k + 1) * P, :]
                    ).then_inc(dw, 16)
                e.dma_start(out=b1sb[:, :], in_=b1[:, :]).then_inc(dw, 16)
                e.dma_start(out=b2sb[:, :], in_=b2[:, :]).then_inc(dw, 16)

            @block.vector
            def _(e):
                # spatial pooling: sum over the 64 spatial positions
                for i in range(NCHUNK):
                    e.wait_ge(dch[i], 16)
                    if i >= NXV:
                        e.wait_ge(s_tp, i - NXV + 1)
                    e.tensor_reduce(
                        xv[i % NXV][:, :],
                        xbuf[i % NBUF][:, :, :],
                        axis=AX.X,
                        op=ALU.add,
                    ).then_inc(s_pool, 1)
                # epilogue: diffs, squares, per-D sums
                e.wait_ge(s_mubias, 2)
                e.wait_ge(s_cp, NCHUNK)
                for m in range(2):
                    e.tensor_sub(dtmp[:, :], muT[:, m, :], yvT[:, m, :])
                    e.tensor_mul(sqd[:, m, :], dtmp[:, :], dtmp[:, :])
                    e.tensor_mul(sqmu[:, m, :], muT[:, m, :], muT[:, m, :])
                    inst = e.tensor_mul(sqy[:, m, :], yvT[:, m, :], yvT[:, m, :])
                    if m == 1:
                        inst.then_inc(s_sq, 1)
                e.tensor_reduce(stat[:, 0:1], yvT[:, 0, :], axis=AX.X, op=ALU.add)
                e.tensor_reduce(stat[:, 1:2], yvT[:, 1, :], axis=AX.X, op=ALU.add)
                e.tensor_reduce(stat[:, 2:3], muT[:, 0, :], axis=AX.X, op=ALU.add)
                e.tensor_reduce(
                    stat[:, 3:4], muT[:, 1, :], axis=AX.X, op=ALU.add
                ).then_inc(s_stat, 1)

            @block.tensor
            def _(e):
                e.wait_ge(s_const, 1)
                for i in range(NCHUNK):
                    e.wait_ge(s_pool, i + 1)
                    if i >= 2:
                        e.wait_ge(s_cp, i - 1)
                    e.transpose(
                        pt[i % 2][:, :], xv[i % NXV][:, :], ident[:, :]
                    ).then_inc(s_tp, 1)
                    if i == NX - 1:
                        # hT[m] = (x_vec @ W1)^T chunks; overlaps y streaming
                        e.wait_ge(s_cp, NX)
                        e.wait_ge(dw, 160)
                        for m in range(4):
                            for k in range(4):
                                mm = e.matmul(
                                    ph[:, m, :],
                                    w1sb[k][:, m * P:(m + 1) * P],
                                    xvT[:, k, :],
                                    start=(k == 0),
                                    stop=(k == 3),
                                )
                        mm.then_inc(s_hmm, 1)
                    if i == NX + 1:
                        e.wait_ge(s_relu, 4)
                        for m in range(2):
                            for k in range(4):
                                mm = e.matmul(
                                    pmu[:, m, :],
                                    w2sb[k][:, m * P:(m + 1) * P],
                                    hT[:, k, :],
                                    start=(k == 0),
                                    stop=(k == 3),
                                )
                        mm.then_inc(s_mumm, 1)
                # partition-sum of the three squared tensors via ones matmul
                e.wait_ge(s_sq, 1)
                for col, sq in ((0, sqd), (1, sqmu), (2, sqy)):
                    for m in range(2):
                        mm = e.matmul(
                            prow[:, col, :],
                            ones[:, :],
                            sq[:, m, :],
                            start=(m == 0),
                            stop=(m == 1),
                        )
                mm.then_inc(s_rows, 1)

            @block.scalar
            def _(e):
                for i in range(NCHUNK):
                    e.wait_ge(s_tp, i + 1)
                    if i < NX:
                        k, half = i // 2, i % 2
                        dst = xvT[half * CH:(half + 1) * CH, k, :]
                    else:
                        j = i - NX
                        k, half = j // 2, j % 2
                        dst = yvT[half * CH:(half + 1) * CH, k, :]
                    # fold the 1/64 spatial mean into the transpose copy (exact)
                    e.activation(
                        dst, pt[i % 2][:, :], ACTF.Copy, scale=1.0 / S
                    ).then_inc(s_cp, 1)
                    if i == NX - 1:
                        e.wait_ge(s_hmm, 1)
                        for m in range(4):
                            e.activation(
                                hT[:, m, :],
                                ph[:, m, :],
                                ACTF.Relu,
                                bias=b1sb[:, m:m + 1],
                            ).then_inc(s_relu, 1)
                    if i == NX + 1:
                        e.wait_ge(s_mumm, 1)
                        for m in range(2):
                            e.activation(
                                muT[:, m, :],
                                pmu[:, m, :],
                                ACTF.Identity,
                                bias=b2sb[:, m:m + 1],
                            ).then_inc(s_mubias, 1)
                e.wait_ge(s_rows, 1)
                e.activation(rows[:, :, :], prow[:, :, :], ACTF.Copy).then_inc(
                    s_rowscp, 1
                )

    return nc


def _get_nc():
    if "nc" not in _CACHE:
        _CACHE["nc"] = build_nc()
    return _CACHE["nc"]


def make_in_maps(x_samples, y_samples, W1, b1, W2, b2):
    xs = np.ascontiguousarray(
        np.asarray(x_samples, np.float32).reshape(N, XC, S)
    )
    ys = np.ascontiguousarray(
        np.asarray(y_samples, np.float32).reshape(N, YC, S)
    )
    w1 = np.ascontiguousarray(np.asarray(W1, np.float32))
    w2 = np.ascontiguousarray(np.asarray(W2, np.float32))
    b1r = np.ascontiguousarray(np.asarray(b1, np.float32).reshape(4, P).T)
    b2r = np.ascontiguousarray(np.asarray(b2, np.float32).reshape(2, P).T)
    in_maps = []
    for c in range(8):
        in_maps.append(
            {
                "x": np.ascontiguousarray(xs[c * P:(c + 1) * P]),
                "y": np.ascontiguousarray(ys[c * P:(c + 1) * P]),
                "w1": w1,
                "b1": b1r,
                "w2": w2,
                "b2": b2r,
            }
        )
    return in_maps


def combine(results):
    A = B = S2 = 0.0
    EyN = np.zeros(YC, np.float64)
    MuN = np.zeros(YC, np.float64)
    for c in range(8):
        vec = results[c]["out_vec"].astype(np.float64)  # (128, 4)
        row = results[c]["out_row"].astype(np.float64)  # (1, 3, 128)
        EyN += np.concatenate([vec[:, 0], vec[:, 1]])
        MuN += np.concatenate([vec[:, 2], vec[:, 3]])
        A += row[0, 0].sum()
        B += row[0, 1].sum()
        S2 += row[0, 2].sum()
    ey = EyN / N
    mu = MuN / N
    loss = -(A / N) / 2.0 + 0.5 * (S2 / N - 2.0 * float(mu @ ey) + B / N)
    return np.float32(loss)


def run(inputs, **kwargs):
    nc = _get_nc()
    in_maps = make_in_maps(**inputs)
    res = run_bass_kernel_spmd(nc, in_maps, core_ids=list(range(8)), **kwargs)
    return combine(res.results), res


def kernel(x_samples, y_samples, W1, b1, W2, b2):
    loss, _ = run(
        dict(
            x_samples=x_samples,
            y_samples=y_samples,
            W1=W1,
            b1=b1,
            W2=W2,
            b2=b2,
        )
    )
    return loss
